# revision 54
# baseline (speedup 1.0000x reference)
"""Multi-head self-attention Trainium2 Bass kernel.

Problem: B=2, S=2048, D=2048, H=16 (head dim 128), fp32, causal mask.
    q = split_heads(x @ Wq.T); k = ...; v = ...
    out = softmax(q k^T / sqrt(hd), causal) v  -> merge heads -> @ Wo.T

Sharding over 8 cores: core c handles batch b=c//4 and head-group hg=c%4
(4 heads = 512 of the 2048 hidden dims).  Each core computes a full
(2048, 2048) partial output (its heads' contribution through Wo columns);
the host sums the 4 partials per batch (row-parallel Wo, reduction on host).

Shard layout choices (host-side, part of the sharding strategy): activations
and weight slices are passed bf16 and contraction-major (pre-transposed), so
every device matmul streams at the bf16 rate with no on-device transposes:
  xt  [D, S]  = x[b].T          wqt/wkt/wvt [D, 512] = W[slice].T
  wot [512, D] = Wo[:, slice].T
All matmul/softmax FLOPs run on device.

Pipeline (PE-roofline oriented; ~306us):
  * DMA-independent warmup matmuls start PE immediately (HAM clock warm).
  * V projection runs d-outer over 8 PSUM banks so PE chases the x^T chunk
    DMAs (wv/x pairs interleaved in d order across both HWDGE rings).
  * V is stored with a ones column per (block, head); the AV matmuls use
    the exp tile E as the stationary operand and stream [V | 1], so the
    softmax denominator accumulates in PSUM col 128 of the same matmul —
    no separate row-sum matmuls.  o' = u * recip(r) via per-partition
    tensor_scalar, then a DMA xbar transpose (sync ring only; ~1us issue
    cost must stay off the ACT queue) writes the [vd, q] layout phase D
    needs.
  * Q/K projections are split into 512-col chunks and interleaved into the
    attention stream to fill the bubbles where C2 chases ACT's exp
    latency; head 3 fills with early phase-D groups instead.
  * Causal mask: DVE tri-multiply on diagonal blocks (latency hidden by
    the chunk fills), except head 3 c2=1 where DVE is busy with phase-D
    copies: there an I-stationary matmul accumulates -60000 into PSUM
    before exp.
  * Output is written bf16 (host sums the 4 partials per batch in fp32).

Built on bacc.Bacc + nc.compile() (legalizes to walrus's 1-wait-per-
instruction limit).  Self-contained: shapes hardcoded, no sibling imports.
"""

import numpy as np
import ml_dtypes

import concourse.bass as bass
import concourse.mybir as mybir
import concourse.tile as tile
from concourse import bacc
from concourse.bass_utils import run_bass_kernel_spmd

F32 = mybir.dt.float32
BF16 = mybir.dt.bfloat16

S = 2048  # sequence length
D = 2048  # model dim
M = 512  # local head dims per core (4 heads x 128)
P = 128  # partitions / head dim
NH = 4  # heads per core
SCALE = float(128) ** -0.5

_CACHED_NC = None


def build_nc():
    nc = bacc.Bacc()

    xt = nc.dram_tensor("xt", [D, S], BF16, kind="ExternalInput")
    wqt = nc.dram_tensor("wqt", [D, M], BF16, kind="ExternalInput")
    wkt = nc.dram_tensor("wkt", [D, M], BF16, kind="ExternalInput")
    wvt = nc.dram_tensor("wvt", [D, M], BF16, kind="ExternalInput")
    wot = nc.dram_tensor("wot", [M, D], BF16, kind="ExternalInput")
    eye_bf = nc.dram_tensor("eye_bf", [P, P], BF16, kind="ExternalInput")
    mtri = nc.dram_tensor("mtri", [P, P], BF16, kind="ExternalInput")
    tri = nc.dram_tensor("tri", [P, P], BF16, kind="ExternalInput")
    out = nc.dram_tensor("out", [S, D], BF16, kind="ExternalOutput")

    xt_r = xt.rearrange("(dh p) s -> p dh s", p=P)  # [128, 16, 2048]
    wqt_r = wqt.rearrange("(dh p) m -> p dh m", p=P)  # [128, 16, 512]
    wkt_r = wkt.rearrange("(dh p) m -> p dh m", p=P)
    wvt_r = wvt.rearrange("(dh p) m -> p dh m", p=P)
    wot_r = wot.rearrange("(h p) e -> p h e", p=P)  # [128, 4, 2048]
    out_r = out.rearrange("(t p) d -> t p d", p=P)

    ND = D // P  # 16 d-chunks
    NT = S // P  # 16 token tiles
    NI = S // 512  # 4 chunks of 512

    with tile.TileContext(nc) as tc:
        with (
            tc.tile_pool(name="const", bufs=1) as constp,
            tc.tile_pool(name="big", bufs=1) as bigp,
            tc.tile_pool(name="vp", bufs=1) as vp,
            tc.tile_pool(name="ot", bufs=4) as otp,
        ):
            eyet = constp.tile([P, P], BF16, tag="eye")
            nc.sync.dma_start(eyet[:], eye_bf[:, :])
            mtrit = constp.tile([P, P], BF16, tag="mtri")
            nc.sync.dma_start(mtrit[:], mtri[:, :])
            trit = constp.tile([P, P], BF16, tag="tri")
            nc.sync.dma_start(trit[:], tri[:, :])
            # DMA-independent warmup operand (memset, no DMA dependency) so
            # PE starts immediately and the HAM clock-gate warms early
            warm_sb = constp.tile([P, 512], BF16, tag="warmsb")
            nc.vector.memset(warm_sb[:], 0.0)

            # Input loads: wv/x chunk pairs in d order across both HWDGE
            # rings so the d-outer V pass can chase chunk arrivals; x's
            # second token-half (only needed by V pass B / projections)
            # queued after all pass-A halves
            xT = bigp.tile([P, ND, S], BF16, tag="xT")
            # V with a ones column appended per (block, head): AV matmuls
            # then produce the softmax denominator in the same accumulation
            VC = 132  # 128 v dims + 1 ones col + pad
            vt2 = vp.tile([P, NT, NH, VC], BF16, tag="V")
            nc.vector.memset(vt2[:, :, :, 128:129], 1.0)
            qkTs = {}

            wvT = vp.tile([P, ND, M], BF16, tag="wvT")
            for dh in range(ND):
                eng = nc.scalar if dh % 2 == 0 else nc.sync
                eng.dma_start(wvT[:, dh, :], wvt_r[:, dh, :])
                eng.dma_start(xT[:, dh, 0:512], xt_r[:, dh, 0:512])
                eng.dma_start(xT[:, dh, 512:1024], xt_r[:, dh, 512:1024])
            for dh in range(ND):
                eng = nc.scalar if dh % 2 == 0 else nc.sync
                eng.dma_start(xT[:, dh, 1024:2048], xt_r[:, dh, 1024:2048])

            # ------- per-head: QK projection interleaved with attention ------
            oTs = [otp.tile([P, S], BF16, tag="oT", name=f"oT{h}") for h in range(NH)]
            CH = 1024
            NC2 = S // CH  # 2
            # SBUF pools for weight tiles / staging live across both the V
            # phase and attention (no ceremony between them)
            bcp = tc.alloc_tile_pool(name="bc", bufs=2)
            cp = tc.alloc_tile_pool(name="cp", bufs=3)
            wts = {}
            wrs = {"q": wqt_r, "k": wkt_r}

            def load_wt(hh):
                for which in ("q", "k"):
                    wt = bcp.tile(
                        [P, ND, P], BF16, tag="wT", bufs=4, name=f"wt{which}{hh}"
                    )
                    nc.sync.dma_start(wt[:], wrs[which][:, :, P * hh : P * (hh + 1)])
                    wts[(hh, which)] = wt

            load_wt(0)
            qkTs[("q", 0)] = bcp.tile([P, S], BF16, tag="qkT", bufs=4, name="qT0")
            # V[p, it, m] = v[it*128+p, m] = sum_d x[i, d] wv[m, d]
            # d-outer over 8 PSUM banks: PE consumes each x chunk as it
            # arrives instead of waiting for the full xT load
            with tc.tile_pool(name="pv", bufs=8, space="PSUM") as pvp:
                warm_ps = pvp.tile([P, 512], F32, tag="vps", name="warm_ps")
                for w in range(9):
                    nc.tensor.matmul(
                        warm_ps[:],
                        lhsT=warm_sb[:, :P],
                        rhs=warm_sb[:],
                        start=True,
                        stop=True,
                        skip_group_check=True,
                    )
                for half in range(2):
                    pss = [
                        pvp.tile([P, 512], F32, tag="vps", name=f"vps{half}_{t}")
                        for t in range(8)
                    ]
                    for d in range(ND - 2):
                        for t in range(8):
                            it = 8 * half + t
                            nc.tensor.matmul(
                                pss[t][:],
                                lhsT=xT[:, d, P * it : P * (it + 1)],
                                rhs=wvT[:, d, :],
                                start=(d == 0),
                                stop=False,
                                skip_group_check=True,
                            )
                    # last two d rounds fused per tile so each tile's copy
                    # fires well before the pass ends: whoever waits on
                    # these (next pass / next pool) isn't gated on the
                    # final matmul + copy latency
                    for t in range(8):
                        it = 8 * half + t
                        for d in (ND - 2, ND - 1):
                            nc.tensor.matmul(
                                pss[t][:],
                                lhsT=xT[:, d, P * it : P * (it + 1)],
                                rhs=wvT[:, d, :],
                                start=False,
                                stop=(d == ND - 1),
                                skip_group_check=True,
                            )
                        dst = vt2[:, it, :, 0:128]
                        src = pss[t][:].rearrange("p (h c) -> p h c", h=NH)
                        if t % 2 == 0:
                            nc.scalar.copy(dst, src)
                        else:
                            nc.vector.tensor_copy(out=dst, in_=src)

                    if half == 0:
                        # first two proj chunks here: they need only the
                        # first token-half of x, filling the wait for pass
                        # B's second-half chunks (pass A is DMA-bound to
                        # its end, so those only start arriving now)
                        for ic in (0, 1):
                            ps = pvp.tile([P, 512], F32, tag="vps", name=f"pj0q{ic}")
                            wt = wts[(0, "q")]
                            for d in range(ND):
                                nc.tensor.matmul(
                                    ps[:],
                                    lhsT=wt[:, d, :],
                                    rhs=xT[:, d, 512 * ic : 512 * (ic + 1)],
                                    start=(d == 0),
                                    stop=(d == ND - 1),
                                )
                            nc.vector.tensor_copy(
                                out=qkTs[("q", 0)][:, 512 * ic : 512 * (ic + 1)],
                                in_=ps[:],
                            )

            # wo^T into the dead wvT slot right after the V passes, far
            # ahead of phase D (off the sync ring's transpose backlog)
            woT = vp.tile([P, NH, D], BF16, tag="wvT", name="woT")
            nc.scalar.dma_start(woT[:, :2, :], wot_r[:, :2, :])
            nc.sync.dma_start(woT[:, 2:, :], wot_r[:, 2:, :])

            with tc.tile_pool(name="ps2", bufs=2, space="PSUM") as psp:
                # Q/K projections split into 512-col chunks and interleaved
                # into the attention stream: they fill PE bubbles where C2
                # would otherwise chase ACT's exp latency.  Chunk order per
                # head: q01, k01, q23, k23 (what each C1 needs, just in
                # time); wt DMAs prefetched 4 chunks ahead.
                chunk_list = []
                for hh in range(NH):
                    for grp in (("q", 0, 1), ("k", 0, 1), ("q", 2, 3), ("k", 2, 3)):
                        which = grp[0]
                        chunk_list += [(hh, which, grp[1]), (hh, which, grp[2])]
                cursor = [2]  # q0 ic0/ic1 were emitted in the V pool scope
                pd_state = [0]

                # Phase D groups (output projection), emitted group-at-a-time
                # so head 3's endgame can interleave them into exp-chase
                # bubbles.  partial[i, e] = sum_m o[i, m] wo[e, m].
                # Copies stay off the ACT queue (strict FIFO) while EXPs are
                # pending: groups below 28 use DVE + sync-ring DMA only.
                def emit_pd(n_target):
                    while pd_state[0] < min(n_target, NT * NI):
                        g = pd_state[0]
                        it, ec = divmod(g, NI)
                        ps = psp.tile(
                            [P, 512], F32, tag="pj", bufs=4, name=f"pd{it}_{ec}"
                        )
                        for hh in range(NH):
                            nc.tensor.matmul(
                                ps[:],
                                lhsT=oTs[hh][:, P * it : P * (it + 1)],
                                rhs=woT[:, hh, 512 * ec : 512 * (ec + 1)],
                                start=(hh == 0),
                                stop=(hh == NH - 1),
                            )
                        ost = cp.tile(
                            [P, 512], BF16, tag="ost", bufs=8, name=f"ost{it}_{ec}"
                        )
                        if g < 28 or g % 2 == 0:
                            nc.vector.tensor_copy(out=ost[:], in_=ps[:])
                        else:
                            nc.scalar.copy(ost[:], ps[:])
                        eng = nc.sync if (g < 28 or g % 2 == 0) else nc.scalar
                        eng.dma_start(out_r[it][:, 512 * ec : 512 * (ec + 1)], ost[:])
                        pd_state[0] += 1

                def emit_chunks(n_target):
                    while cursor[0] < min(n_target, len(chunk_list)):
                        idx = cursor[0]
                        hh, which, ic = chunk_list[idx]
                        if idx % 8 == 4 and hh + 1 < NH:
                            load_wt(hh + 1)
                        if (which, hh) not in qkTs:
                            qkTs[(which, hh)] = bcp.tile(
                                [P, S], BF16, tag="qkT", bufs=4, name=f"{which}T{hh}"
                            )
                        dst = qkTs[(which, hh)]
                        wt = wts[(hh, which)]
                        ps = psp.tile(
                            [P, 512], F32, tag="pj", bufs=4, name=f"pj{hh}{which}{ic}"
                        )
                        for d in range(ND):
                            nc.tensor.matmul(
                                ps[:],
                                lhsT=wt[:, d, :],
                                rhs=xT[:, d, 512 * ic : 512 * (ic + 1)],
                                start=(d == 0),
                                stop=(d == ND - 1),
                            )
                        nc.vector.tensor_copy(
                            out=dst[:, 512 * ic : 512 * (ic + 1)], in_=ps[:]
                        )
                        cursor[0] += 1

                for h in range(NH):
                    for c2 in range(NC2):
                        i0 = CH * c2
                        njb = 8 * c2 + 8
                        # this C1 needs q chunks [2c2, 2c2+2) and k chunks
                        # [0, 2+2c2) of head h
                        emit_chunks(8 * h + 4 + 4 * c2)
                        # C1: scores -> exp into SBUF-staged E tiles
                        e8s = [
                            cp.tile(
                                [P, 8, CH], BF16, tag="E8", bufs=3, name=f"e8_{h}_{c2}_{g}"
                            )
                            for g in range(njb // 8)
                        ]
                        # head 3 endgame keeps the causal mask on PE (an
                        # I-stationary matmul accumulating -60000 before
                        # exp) because DVE is saturated with phase-D copies
                        # there; elsewhere the mask is a DVE tri-multiply
                        # whose latency hides behind the proj-chunk fills.
                        pe_mask = h == 3 and c2 == 1
                        for jb in range(njb):
                            i_start = max(0, P * jb - i0)
                            sc = psp.tile([P, CH], F32, tag="sc")
                            t = jb - 8 * c2
                            rest = i_start
                            if t >= 0 and pe_mask:
                                d0 = P * t
                                nc.tensor.matmul(
                                    sc[:, d0 : d0 + P],
                                    lhsT=eyet[:],
                                    rhs=mtrit[:],
                                    start=True,
                                    stop=False,
                                )
                                nc.tensor.matmul(
                                    sc[:, d0 : d0 + P],
                                    lhsT=qkTs[("k", h)][:, P * jb : P * (jb + 1)],
                                    rhs=qkTs[("q", h)][:, i0 + d0 : i0 + d0 + P],
                                    start=False,
                                    stop=True,
                                )
                                rest = d0 + P
                            segs = [
                                (s0, s1)
                                for s0, s1 in (
                                    (rest, 512),
                                    (max(512, rest), CH),
                                )
                                if s0 < s1
                            ]
                            for s0, s1 in segs:
                                nc.tensor.matmul(
                                    sc[:, s0:s1],
                                    lhsT=qkTs[("k", h)][:, P * jb : P * (jb + 1)],
                                    rhs=qkTs[("q", h)][:, i0 + s0 : i0 + s1],
                                    start=True,
                                    stop=True,
                                )
                            et = e8s[jb // 8]
                            nc.scalar.activation(
                                et[:, jb % 8, i_start:CH],
                                sc[:, i_start:CH],
                                mybir.ActivationFunctionType.Exp,
                                scale=SCALE,
                            )
                            if t >= 0 and not pe_mask:
                                nc.vector.tensor_tensor(
                                    et[:, jb % 8, P * t : P * (t + 1)],
                                    et[:, jb % 8, P * t : P * (t + 1)],
                                    trit[:],
                                    mybir.AluOpType.mult,
                                )
                        # fill the C2 exp-chase bubble with proj chunks
                        # (head 3 has none left: use phase D's first half,
                        # whose c2=0 oT inputs are complete)
                        emit_chunks(cursor[0] + (2 if c2 == 0 else 4))
                        if h == 3 and c2 == 1:
                            emit_pd(28)
                        # C2: per q-tile, E-stationary AV over key blocks;
                        # the ones column of vt2 accumulates the softmax
                        # denominator into pu[:, 128] in the same matmuls.
                        # o' comes out [q, vd]; DMA xbar transpose writes
                        # the [vd, q] layout phase D needs.
                        for t in range(8):
                            jbt = 8 * c2 + t
                            pu = psp.tile(
                                [P, 136], F32, tag="pj", bufs=4, name=f"pu{h}{c2}{t}"
                            )
                            for jb in range(jbt + 1):
                                et = e8s[jb // 8]
                                nc.tensor.matmul(
                                    pu[:, 0:129],
                                    lhsT=et[:, jb % 8, P * t : P * (t + 1)],
                                    rhs=vt2[:, jb, h, 0:129],
                                    start=(jb == 0),
                                    stop=(jb == jbt),
                                    skip_group_check=True,
                                )
                            rinv = cp.tile([P, 1], F32, tag="rinv", bufs=6)
                            nc.vector.reciprocal_approx_fast(rinv[:], pu[:, 128:129])
                            o2 = cp.tile([P, P], BF16, tag="o2", bufs=6)
                            nc.vector.tensor_scalar(
                                o2[:],
                                pu[:, 0:128],
                                rinv[:],
                                None,
                                mybir.AluOpType.mult,
                            )
                            # transpose issue costs ~1us on the issuing
                            # engine's queue: keep it off ACT (EXP feeder)
                            nc.sync.dma_start_transpose(
                                oTs[h][:, i0 + P * t : i0 + P * (t + 1)], o2[:]
                            )
                            if h == 3 and c2 == 1 and t >= 1:
                                # it = 7 needs nothing new; it = 8+t' waits
                                # on this c2's transpose t' (2-lag gives the
                                # sync ring time to finish it)
                                emit_pd(32 + 4 * (t - 1))
                        if c2 == 1:
                            # keep PE fed while next head's C1 exps spin up
                            emit_chunks(cursor[0] + 2)

                # ---------- Phase D: remaining output-projection tiles ------
                emit_pd(NT * NI)

            cp.release()
            bcp.release()

    nc.compile()
    return nc


def make_in_maps(x, Wq, Wk, Wv, Wo):
    bf = ml_dtypes.bfloat16
    eye_bf = np.eye(P, dtype=bf)
    jj, ii = np.meshgrid(np.arange(P), np.arange(P), indexing="ij")
    # mtri[k, q] = -60000 where k > q (causal-blocked), else 0
    mtri = np.where(jj > ii, np.float32(-60000.0), np.float32(0.0)).astype(bf)
    tri = (jj <= ii).astype(bf)  # keep k <= q

    xtb = [np.ascontiguousarray(x[0].T).astype(bf), np.ascontiguousarray(x[1].T).astype(bf)]
    in_maps = []
    for c in range(8):
        b, hg = c // 4, c % 4
        sl = slice(M * hg, M * (hg + 1))
        in_maps.append(
            {
                "xt": xtb[b],
                "wqt": np.ascontiguousarray(Wq[sl].T).astype(bf),
                "wkt": np.ascontiguousarray(Wk[sl].T).astype(bf),
                "wvt": np.ascontiguousarray(Wv[sl].T).astype(bf),
                "wot": np.ascontiguousarray(Wo[:, sl].T).astype(bf),
                "eye_bf": eye_bf,
                "mtri": mtri,
                "tri": tri,
            }
        )
    return in_maps


def kernel(x, mask, Wq, Wk, Wv, Wo, _trace=False):
    global _CACHED_NC
    x = np.asarray(x, dtype=np.float32)
    Wq = np.asarray(Wq, dtype=np.float32)
    Wk = np.asarray(Wk, dtype=np.float32)
    Wv = np.asarray(Wv, dtype=np.float32)
    Wo = np.asarray(Wo, dtype=np.float32)
    if _CACHED_NC is None:
        _CACHED_NC = build_nc()
    nc = _CACHED_NC
    in_maps = make_in_maps(x, Wq, Wk, Wv, Wo)
    res = run_bass_kernel_spmd(nc, in_maps, list(range(8)), trace=_trace)
    outs = [np.asarray(r["out"], dtype=np.float32) for r in res.results]
    full = np.empty((2, S, D), dtype=np.float32)
    for b in range(2):
        full[b] = outs[4 * b] + outs[4 * b + 1] + outs[4 * b + 2] + outs[4 * b + 3]
    kernel.last_exec_time_ns = res.exec_time_ns
    return full



# revision 55
# speedup vs baseline: 1.0135x; 1.0135x over previous
"""Multi-head self-attention Trainium2 Bass kernel.

Problem: B=2, S=2048, D=2048, H=16 (head dim 128), fp32, causal mask.
    q = split_heads(x @ Wq.T); k = ...; v = ...
    out = softmax(q k^T / sqrt(hd), causal) v  -> merge heads -> @ Wo.T

Sharding over 8 cores: core c handles batch b=c//4 and head-group hg=c%4
(4 heads = 512 of the 2048 hidden dims).  Each core computes a full
(2048, 2048) partial output (its heads' contribution through Wo columns);
the host sums the 4 partials per batch (row-parallel Wo, reduction on host).

Shard layout choices (host-side, part of the sharding strategy): activations
and weight slices are passed bf16 and contraction-major (pre-transposed), so
every device matmul streams at the bf16 rate with no on-device transposes:
  xt  [D, S]  = x[b].T          wqt/wkt/wvt [D, 512] = W[slice].T
  wot [512, D] = Wo[:, slice].T
All matmul/softmax FLOPs run on device.

Pipeline (PE-roofline oriented; ~306us):
  * DMA-independent warmup matmuls start PE immediately (HAM clock warm).
  * V projection runs d-outer over 8 PSUM banks so PE chases the x^T chunk
    DMAs (wv/x pairs interleaved in d order across both HWDGE rings).
  * V is stored with a ones column per (block, head); the AV matmuls use
    the exp tile E as the stationary operand and stream [V | 1], so the
    softmax denominator accumulates in PSUM col 128 of the same matmul —
    no separate row-sum matmuls.  o' = u * recip(r) via per-partition
    tensor_scalar, then a DMA xbar transpose (sync ring only; ~1us issue
    cost must stay off the ACT queue) writes the [vd, q] layout phase D
    needs.
  * Q/K projections are split into 512-col chunks and interleaved into the
    attention stream to fill the bubbles where C2 chases ACT's exp
    latency; head 3 fills with early phase-D groups instead.
  * Causal mask: DVE tri-multiply on diagonal blocks (latency hidden by
    the chunk fills), except head 3 c2=1 where DVE is busy with phase-D
    copies: there an I-stationary matmul accumulates -60000 into PSUM
    before exp.
  * Output is written bf16 (host sums the 4 partials per batch in fp32).

Built on bacc.Bacc + nc.compile() (legalizes to walrus's 1-wait-per-
instruction limit).  Self-contained: shapes hardcoded, no sibling imports.
"""

import numpy as np
import ml_dtypes

import concourse.bass as bass
import concourse.mybir as mybir
import concourse.tile as tile
from concourse import bacc
from concourse.bass_utils import run_bass_kernel_spmd

F32 = mybir.dt.float32
BF16 = mybir.dt.bfloat16

S = 2048  # sequence length
D = 2048  # model dim
M = 512  # local head dims per core (4 heads x 128)
P = 128  # partitions / head dim
NH = 4  # heads per core
SCALE = float(128) ** -0.5

_CACHED_NC = None


def build_nc():
    nc = bacc.Bacc()

    xt = nc.dram_tensor("xt", [D, S], BF16, kind="ExternalInput")
    wqt = nc.dram_tensor("wqt", [D, M], BF16, kind="ExternalInput")
    wkt = nc.dram_tensor("wkt", [D, M], BF16, kind="ExternalInput")
    wvt = nc.dram_tensor("wvt", [D, M], BF16, kind="ExternalInput")
    wot = nc.dram_tensor("wot", [M, D], BF16, kind="ExternalInput")
    eye_bf = nc.dram_tensor("eye_bf", [P, P], BF16, kind="ExternalInput")
    mtri = nc.dram_tensor("mtri", [P, P], BF16, kind="ExternalInput")
    tri = nc.dram_tensor("tri", [P, P], BF16, kind="ExternalInput")
    out = nc.dram_tensor("out", [S, D], BF16, kind="ExternalOutput")

    xt_r = xt.rearrange("(dh p) s -> p dh s", p=P)  # [128, 16, 2048]
    wqt_r = wqt.rearrange("(dh p) m -> p dh m", p=P)  # [128, 16, 512]
    wkt_r = wkt.rearrange("(dh p) m -> p dh m", p=P)
    wvt_r = wvt.rearrange("(dh p) m -> p dh m", p=P)
    wot_r = wot.rearrange("(h p) e -> p h e", p=P)  # [128, 4, 2048]
    out_r = out.rearrange("(t p) d -> t p d", p=P)

    ND = D // P  # 16 d-chunks
    NT = S // P  # 16 token tiles
    NI = S // 512  # 4 chunks of 512

    with tile.TileContext(nc) as tc:
        with (
            tc.tile_pool(name="const", bufs=1) as constp,
            tc.tile_pool(name="big", bufs=1) as bigp,
            tc.tile_pool(name="vp", bufs=1) as vp,
            tc.tile_pool(name="ot", bufs=4) as otp,
        ):
            eyet = constp.tile([P, P], BF16, tag="eye")
            nc.sync.dma_start(eyet[:], eye_bf[:, :])
            mtrit = constp.tile([P, P], BF16, tag="mtri")
            nc.sync.dma_start(mtrit[:], mtri[:, :])
            trit = constp.tile([P, P], BF16, tag="tri")
            nc.sync.dma_start(trit[:], tri[:, :])
            # DMA-independent warmup operand (memset, no DMA dependency) so
            # PE starts immediately and the HAM clock-gate warms early
            warm_sb = constp.tile([P, 512], BF16, tag="warmsb")
            nc.vector.memset(warm_sb[:], 0.0)

            # Input loads: wv/x chunk pairs in d order across both HWDGE
            # rings so the d-outer V pass can chase chunk arrivals; x's
            # second token-half (only needed by V pass B / projections)
            # queued after all pass-A halves
            xT = bigp.tile([P, ND, S], BF16, tag="xT")
            # V with a ones column appended per (block, head): AV matmuls
            # then produce the softmax denominator in the same accumulation
            VC = 132  # 128 v dims + 1 ones col + pad
            vt2 = vp.tile([P, NT, NH, VC], BF16, tag="V")
            nc.vector.memset(vt2[:, :, :, 128:129], 1.0)
            qkTs = {}

            # SBUF pools for weight tiles / staging live across both the V
            # phase and attention (no ceremony between them)
            bcp = tc.alloc_tile_pool(name="bc", bufs=2)
            cp = tc.alloc_tile_pool(name="cp", bufs=3)
            wts = {}
            wrs = {"q": wqt_r, "k": wkt_r}

            def load_wt(hh):
                for which in ("q", "k"):
                    wt = bcp.tile(
                        [P, ND, P], BF16, tag="wT", bufs=4, name=f"wt{which}{hh}"
                    )
                    nc.sync.dma_start(wt[:], wrs[which][:, :, P * hh : P * (hh + 1)])
                    wts[(hh, which)] = wt

            wvT = vp.tile([P, ND, M], BF16, tag="wvT")
            for dh in range(ND):
                eng = nc.scalar if dh % 2 == 0 else nc.sync
                eng.dma_start(wvT[:, dh, :], wvt_r[:, dh, :])
                eng.dma_start(xT[:, dh, 0:512], xt_r[:, dh, 0:512])
                eng.dma_start(xT[:, dh, 512:1024], xt_r[:, dh, 512:1024])
            # head-0 weights before the second token-half so the proj
            # chunks between the V passes aren't starved
            load_wt(0)
            for dh in range(ND):
                eng = nc.scalar if dh % 2 == 0 else nc.sync
                eng.dma_start(xT[:, dh, 1024:2048], xt_r[:, dh, 1024:2048])

            # ------- per-head: QK projection interleaved with attention ------
            oTs = [otp.tile([P, S], BF16, tag="oT", name=f"oT{h}") for h in range(NH)]
            CH = 1024
            NC2 = S // CH  # 2
            qkTs[("q", 0)] = bcp.tile([P, S], BF16, tag="qkT", bufs=4, name="qT0")
            # V[p, it, m] = v[it*128+p, m] = sum_d x[i, d] wv[m, d]
            # d-outer over 8 PSUM banks: PE consumes each x chunk as it
            # arrives instead of waiting for the full xT load
            with tc.tile_pool(name="pv", bufs=8, space="PSUM") as pvp:
                warm_ps = pvp.tile([P, 512], F32, tag="vps", name="warm_ps")
                for w in range(9):
                    nc.tensor.matmul(
                        warm_ps[:],
                        lhsT=warm_sb[:, :P],
                        rhs=warm_sb[:],
                        start=True,
                        stop=True,
                        skip_group_check=True,
                    )
                for half in range(2):
                    pss = [
                        pvp.tile([P, 512], F32, tag="vps", name=f"vps{half}_{t}")
                        for t in range(8)
                    ]
                    for d in range(ND - 2):
                        for t in range(8):
                            it = 8 * half + t
                            nc.tensor.matmul(
                                pss[t][:],
                                lhsT=xT[:, d, P * it : P * (it + 1)],
                                rhs=wvT[:, d, :],
                                start=(d == 0),
                                stop=False,
                                skip_group_check=True,
                            )
                    # last two d rounds fused per tile so each tile's copy
                    # fires well before the pass ends: whoever waits on
                    # these (next pass / next pool) isn't gated on the
                    # final matmul + copy latency
                    for t in range(8):
                        it = 8 * half + t
                        for d in (ND - 2, ND - 1):
                            nc.tensor.matmul(
                                pss[t][:],
                                lhsT=xT[:, d, P * it : P * (it + 1)],
                                rhs=wvT[:, d, :],
                                start=False,
                                stop=(d == ND - 1),
                                skip_group_check=True,
                            )
                        dst = vt2[:, it, :, 0:128]
                        src = pss[t][:].rearrange("p (h c) -> p h c", h=NH)
                        if t % 2 == 0:
                            nc.scalar.copy(dst, src)
                        else:
                            nc.vector.tensor_copy(out=dst, in_=src)

                    if half == 0:
                        # first two proj chunks here: they need only the
                        # first token-half of x, filling the wait for pass
                        # B's second-half chunks (pass A is DMA-bound to
                        # its end, so those only start arriving now)
                        for ic in (0, 1):
                            ps = pvp.tile([P, 512], F32, tag="vps", name=f"pj0q{ic}")
                            wt = wts[(0, "q")]
                            for d in range(ND):
                                nc.tensor.matmul(
                                    ps[:],
                                    lhsT=wt[:, d, :],
                                    rhs=xT[:, d, 512 * ic : 512 * (ic + 1)],
                                    start=(d == 0),
                                    stop=(d == ND - 1),
                                )
                            nc.vector.tensor_copy(
                                out=qkTs[("q", 0)][:, 512 * ic : 512 * (ic + 1)],
                                in_=ps[:],
                            )

            # wo^T into the dead wvT slot right after the V passes, far
            # ahead of phase D (off the sync ring's transpose backlog)
            woT = vp.tile([P, NH, D], BF16, tag="wvT", name="woT")
            nc.scalar.dma_start(woT[:, :2, :], wot_r[:, :2, :])
            nc.sync.dma_start(woT[:, 2:, :], wot_r[:, 2:, :])

            with tc.tile_pool(name="ps2", bufs=2, space="PSUM") as psp:
                # Q/K projections split into 512-col chunks and interleaved
                # into the attention stream: they fill PE bubbles where C2
                # would otherwise chase ACT's exp latency.  Chunk order per
                # head: q01, k01, q23, k23 (what each C1 needs, just in
                # time); wt DMAs prefetched 4 chunks ahead.
                chunk_list = []
                for hh in range(NH):
                    for grp in (("q", 0, 1), ("k", 0, 1), ("q", 2, 3), ("k", 2, 3)):
                        which = grp[0]
                        chunk_list += [(hh, which, grp[1]), (hh, which, grp[2])]
                cursor = [2]  # q0 ic0/ic1 were emitted in the V pool scope
                pd_state = [0]

                # Phase D groups (output projection), emitted group-at-a-time
                # so head 3's endgame can interleave them into exp-chase
                # bubbles.  partial[i, e] = sum_m o[i, m] wo[e, m].
                # Copies stay off the ACT queue (strict FIFO) while EXPs are
                # pending: groups below 28 use DVE + sync-ring DMA only.
                def emit_pd(n_target):
                    while pd_state[0] < min(n_target, NT * NI):
                        g = pd_state[0]
                        it, ec = divmod(g, NI)
                        ps = psp.tile(
                            [P, 512], F32, tag="pj", bufs=4, name=f"pd{it}_{ec}"
                        )
                        for hh in range(NH):
                            nc.tensor.matmul(
                                ps[:],
                                lhsT=oTs[hh][:, P * it : P * (it + 1)],
                                rhs=woT[:, hh, 512 * ec : 512 * (ec + 1)],
                                start=(hh == 0),
                                stop=(hh == NH - 1),
                            )
                        ost = cp.tile(
                            [P, 512], BF16, tag="ost", bufs=8, name=f"ost{it}_{ec}"
                        )
                        if g < 28 or g % 2 == 0:
                            nc.vector.tensor_copy(out=ost[:], in_=ps[:])
                        else:
                            nc.scalar.copy(ost[:], ps[:])
                        eng = nc.sync if (g < 28 or g % 2 == 0) else nc.scalar
                        eng.dma_start(out_r[it][:, 512 * ec : 512 * (ec + 1)], ost[:])
                        pd_state[0] += 1

                def emit_chunks(n_target):
                    while cursor[0] < min(n_target, len(chunk_list)):
                        idx = cursor[0]
                        hh, which, ic = chunk_list[idx]
                        if idx % 8 == 4 and hh + 1 < NH:
                            load_wt(hh + 1)
                        if (which, hh) not in qkTs:
                            qkTs[(which, hh)] = bcp.tile(
                                [P, S], BF16, tag="qkT", bufs=4, name=f"{which}T{hh}"
                            )
                        dst = qkTs[(which, hh)]
                        wt = wts[(hh, which)]
                        ps = psp.tile(
                            [P, 512], F32, tag="pj", bufs=4, name=f"pj{hh}{which}{ic}"
                        )
                        for d in range(ND):
                            nc.tensor.matmul(
                                ps[:],
                                lhsT=wt[:, d, :],
                                rhs=xT[:, d, 512 * ic : 512 * (ic + 1)],
                                start=(d == 0),
                                stop=(d == ND - 1),
                            )
                        nc.vector.tensor_copy(
                            out=dst[:, 512 * ic : 512 * (ic + 1)], in_=ps[:]
                        )
                        cursor[0] += 1

                for h in range(NH):
                    for c2 in range(NC2):
                        i0 = CH * c2
                        njb = 8 * c2 + 8
                        # this C1 needs q chunks [2c2, 2c2+2) and k chunks
                        # [0, 2+2c2) of head h
                        emit_chunks(8 * h + 4 + 4 * c2)
                        # C1: scores -> exp into SBUF-staged E tiles
                        e8s = [
                            cp.tile(
                                [P, 8, CH], BF16, tag="E8", bufs=3, name=f"e8_{h}_{c2}_{g}"
                            )
                            for g in range(njb // 8)
                        ]
                        # head 3 endgame keeps the causal mask on PE (an
                        # I-stationary matmul accumulating -60000 before
                        # exp) because DVE is saturated with phase-D copies
                        # there; elsewhere the mask is a DVE tri-multiply
                        # whose latency hides behind the proj-chunk fills.
                        pe_mask = h == 3 and c2 == 1
                        for jb in range(njb):
                            i_start = max(0, P * jb - i0)
                            sc = psp.tile([P, CH], F32, tag="sc")
                            t = jb - 8 * c2
                            rest = i_start
                            if t >= 0 and pe_mask:
                                d0 = P * t
                                nc.tensor.matmul(
                                    sc[:, d0 : d0 + P],
                                    lhsT=eyet[:],
                                    rhs=mtrit[:],
                                    start=True,
                                    stop=False,
                                )
                                nc.tensor.matmul(
                                    sc[:, d0 : d0 + P],
                                    lhsT=qkTs[("k", h)][:, P * jb : P * (jb + 1)],
                                    rhs=qkTs[("q", h)][:, i0 + d0 : i0 + d0 + P],
                                    start=False,
                                    stop=True,
                                )
                                rest = d0 + P
                            segs = [
                                (s0, s1)
                                for s0, s1 in (
                                    (rest, 512),
                                    (max(512, rest), CH),
                                )
                                if s0 < s1
                            ]
                            for s0, s1 in segs:
                                nc.tensor.matmul(
                                    sc[:, s0:s1],
                                    lhsT=qkTs[("k", h)][:, P * jb : P * (jb + 1)],
                                    rhs=qkTs[("q", h)][:, i0 + s0 : i0 + s1],
                                    start=True,
                                    stop=True,
                                )
                            et = e8s[jb // 8]
                            nc.scalar.activation(
                                et[:, jb % 8, i_start:CH],
                                sc[:, i_start:CH],
                                mybir.ActivationFunctionType.Exp,
                                scale=SCALE,
                            )
                            if t >= 0 and not pe_mask:
                                nc.vector.tensor_tensor(
                                    et[:, jb % 8, P * t : P * (t + 1)],
                                    et[:, jb % 8, P * t : P * (t + 1)],
                                    trit[:],
                                    mybir.AluOpType.mult,
                                )
                        # fill the C2 exp-chase bubble with proj chunks
                        # (head 3 has none left: use phase D's first half,
                        # whose c2=0 oT inputs are complete)
                        emit_chunks(cursor[0] + (2 if c2 == 0 else 4))
                        if h == 3 and c2 == 1:
                            emit_pd(28)
                        # C2: per q-tile, E-stationary AV over key blocks;
                        # the ones column of vt2 accumulates the softmax
                        # denominator into pu[:, 128] in the same matmuls.
                        # o' comes out [q, vd]; DMA xbar transpose writes
                        # the [vd, q] layout phase D needs.
                        for t in range(8):
                            jbt = 8 * c2 + t
                            pu = psp.tile(
                                [P, 136], F32, tag="pj", bufs=4, name=f"pu{h}{c2}{t}"
                            )
                            for jb in range(jbt + 1):
                                et = e8s[jb // 8]
                                nc.tensor.matmul(
                                    pu[:, 0:129],
                                    lhsT=et[:, jb % 8, P * t : P * (t + 1)],
                                    rhs=vt2[:, jb, h, 0:129],
                                    start=(jb == 0),
                                    stop=(jb == jbt),
                                    skip_group_check=True,
                                )
                            rinv = cp.tile([P, 1], F32, tag="rinv", bufs=6)
                            nc.vector.reciprocal_approx_fast(rinv[:], pu[:, 128:129])
                            o2 = cp.tile([P, P], BF16, tag="o2", bufs=6)
                            nc.vector.tensor_scalar(
                                o2[:],
                                pu[:, 0:128],
                                rinv[:],
                                None,
                                mybir.AluOpType.mult,
                            )
                            # transpose issue costs ~1us on the issuing
                            # engine's queue: keep it off ACT (EXP feeder)
                            nc.sync.dma_start_transpose(
                                oTs[h][:, i0 + P * t : i0 + P * (t + 1)], o2[:]
                            )
                            if h == 3 and c2 == 1 and t >= 1:
                                # it = 7 needs nothing new; it = 8+t' waits
                                # on this c2's transpose t' (2-lag gives the
                                # sync ring time to finish it)
                                emit_pd(32 + 4 * (t - 1))
                        if c2 == 1:
                            # keep PE fed while next head's C1 exps spin up
                            emit_chunks(cursor[0] + 2)

                # ---------- Phase D: remaining output-projection tiles ------
                emit_pd(NT * NI)

            cp.release()
            bcp.release()

    nc.compile()
    return nc


def make_in_maps(x, Wq, Wk, Wv, Wo):
    bf = ml_dtypes.bfloat16
    eye_bf = np.eye(P, dtype=bf)
    jj, ii = np.meshgrid(np.arange(P), np.arange(P), indexing="ij")
    # mtri[k, q] = -60000 where k > q (causal-blocked), else 0
    mtri = np.where(jj > ii, np.float32(-60000.0), np.float32(0.0)).astype(bf)
    tri = (jj <= ii).astype(bf)  # keep k <= q

    xtb = [np.ascontiguousarray(x[0].T).astype(bf), np.ascontiguousarray(x[1].T).astype(bf)]
    in_maps = []
    for c in range(8):
        b, hg = c // 4, c % 4
        sl = slice(M * hg, M * (hg + 1))
        in_maps.append(
            {
                "xt": xtb[b],
                "wqt": np.ascontiguousarray(Wq[sl].T).astype(bf),
                "wkt": np.ascontiguousarray(Wk[sl].T).astype(bf),
                "wvt": np.ascontiguousarray(Wv[sl].T).astype(bf),
                "wot": np.ascontiguousarray(Wo[:, sl].T).astype(bf),
                "eye_bf": eye_bf,
                "mtri": mtri,
                "tri": tri,
            }
        )
    return in_maps


def kernel(x, mask, Wq, Wk, Wv, Wo, _trace=False):
    global _CACHED_NC
    x = np.asarray(x, dtype=np.float32)
    Wq = np.asarray(Wq, dtype=np.float32)
    Wk = np.asarray(Wk, dtype=np.float32)
    Wv = np.asarray(Wv, dtype=np.float32)
    Wo = np.asarray(Wo, dtype=np.float32)
    if _CACHED_NC is None:
        _CACHED_NC = build_nc()
    nc = _CACHED_NC
    in_maps = make_in_maps(x, Wq, Wk, Wv, Wo)
    res = run_bass_kernel_spmd(nc, in_maps, list(range(8)), trace=_trace)
    outs = [np.asarray(r["out"], dtype=np.float32) for r in res.results]
    full = np.empty((2, S, D), dtype=np.float32)
    for b in range(2):
        full[b] = outs[4 * b] + outs[4 * b + 1] + outs[4 * b + 2] + outs[4 * b + 3]
    kernel.last_exec_time_ns = res.exec_time_ns
    return full

